# revision 6
# baseline (speedup 1.0000x reference)
"""Trainium2 Bass kernel for nn_CDE v2: bf16 feature-major pipeline.

Design vs v1:
- Nc = ceil(nact/8) rounded up (132 for the 1044-active case) instead of 256:
  streams scale with actual row count.
- All matmuls bf16 (1 cyc/row at any free width; f32r pays 4x below 256).
- L3 feature-major per-d (W3 stationary, d-major permutation): streams scale
  with Nc, bias folds into the per-partition tanh bias, and the einsum
  reduction moves to one strided DVE tensor_reduce -- no transpose matmuls,
  no bias matmuls.
- dX/dt rows are PE-broadcast into PSUM ([1,128] ones stationary) per stage;
  emitted as next-stage filler behind L3 so the PE stays busy during the
  tanh/mult/reduce tail.
- RK z-update restructured: exactly one fused scalar_tensor_tensor on the
  critical path per sub-stage; all other partials run off-path on Pool.
- PSUM: tag "pp" (mlp m-chunks + L3 per-d, bufs=4) + tag "pdx" (dx broadcast,
  bufs=4) = 8 banks.
"""
import os
import sys
import types

for _p in ("/opt/trn_rl_repo", "/root/.axon_site/_ro/trn_rl_repo"):
    if os.path.isdir(_p) and _p not in sys.path:
        sys.path.insert(0, _p)

if "antenv.axon_hooks" not in sys.modules:
    _m = types.ModuleType("antenv.axon_hooks")
    _hook = [None]

    def _set(hook):
        _hook[0] = hook

    def _get():
        if _hook[0] is None:
            try:
                from trn_agent_boot.trn_boot import _ntff_profile_via_ctypes
                _hook[0] = _ntff_profile_via_ctypes("/opt/axon/libaxon_pjrt.so")
            except Exception:
                pass
        return _hook[0]

    _m.set_axon_ntff_profile_hook = _set
    _m.get_axon_ntff_profile_hook = _get
    sys.modules["antenv.axon_hooks"] = _m

import numpy as np

N_CORES = 8
T, D, E, H = 16, 10, 128, 512
F3 = E * D
N_STEPS = T - 1
N_STAGES = 4 * N_STEPS  # 60
# dx-broadcast d-groups per stage (each tile <= 1 psum bank at Nc<=160)
DX_GROUPS = [(0, 3), (3, 3), (6, 3), (9, 1)]

last_results = None


def spline_stage_matrix(t):
    """C60 (60,16): row 4j+r maps 16 knots of a scalar series to the spline
    derivative at RK stage r of step j.  Also returns h (15,)."""
    t = np.asarray(t, np.float64)
    Tn = len(t)
    h = np.diff(t)
    A = np.zeros((Tn, Tn))
    A[0, 0] = 1.0
    A[-1, -1] = 1.0
    for i in range(1, Tn - 1):
        A[i, i - 1] = h[i - 1]
        A[i, i] = 2.0 * (h[i - 1] + h[i])
        A[i, i + 1] = h[i]
    R = np.zeros((Tn, Tn))
    for i in range(1, Tn - 1):
        R[i, i - 1] = 6.0 / h[i - 1]
        R[i, i] = -6.0 / h[i - 1] - 6.0 / h[i]
        R[i, i + 1] = 6.0 / h[i]
    S = np.linalg.solve(A, R)
    Iden = np.eye(Tn)
    rows = []
    for j in range(Tn - 1):
        hs = h[j]
        for u_frac in (0.0, 1.0 / 3.0, 2.0 / 3.0, 1.0):
            s = t[j + 1] if u_frac == 1.0 else t[j] + u_frac * hs
            i = int(np.clip(np.searchsorted(t, s, side="right") - 1, 0, Tn - 2))
            u = s - t[i]
            b_row = (Iden[i + 1] - Iden[i]) / h[i] - h[i] * (2.0 * S[i] + S[i + 1]) / 6.0
            rows.append(b_row + u * S[i] + (u * u) / (2.0 * h[i]) * (S[i + 1] - S[i]))
    return np.asarray(rows), h


def w3_perm():
    """Permutation so W3p[f'] = W3[e*10+d] with f' = d*128+e (d-major)."""
    fp = np.arange(F3)
    return (fp % E) * D + fp // E


def build_bass2(Nc, h):
    import concourse.bass as bass
    import concourse.bacc as bacc
    import concourse.tile as tile
    import concourse.mybir as mybir

    F32 = mybir.dt.float32
    F32R = mybir.dt.float32r
    BF16 = mybir.dt.bfloat16
    AF = mybir.ActivationFunctionType
    ALU = mybir.AluOpType
    AX = mybir.AxisListType

    nc = bacc.Bacc("TRN2", target_bir_lowering=False)

    d_xt0 = nc.dram_tensor("x_t0", [128, Nc], BF16, kind="ExternalInput")
    d_dxh = nc.dram_tensor("dxh", [D, N_STAGES * Nc], BF16, kind="ExternalInput")
    d_wemb = nc.dram_tensor("w_embt", [128, E], BF16, kind="ExternalInput")
    d_bemb = nc.dram_tensor("b_emb", [E, 1], F32, kind="ExternalInput")
    d_w0 = nc.dram_tensor("w0t", [E, H], BF16, kind="ExternalInput")
    d_w1 = nc.dram_tensor("w1t", [H, H], BF16, kind="ExternalInput")
    d_w2 = nc.dram_tensor("w2t", [H, H], BF16, kind="ExternalInput")
    d_w3 = nc.dram_tensor("w3pt", [H, F3], BF16, kind="ExternalInput")
    d_b012 = nc.dram_tensor("b012", [E, 12], F32, kind="ExternalInput")
    d_b3seg = nc.dram_tensor("b3seg", [128, 4 * E], BF16, kind="ExternalInput")
    d_sel3 = nc.dram_tensor("sel3", [128, 3 * Nc], BF16, kind="ExternalInput")
    d_out = nc.dram_tensor("zout", [E, Nc], F32, kind="ExternalOutput")

    with tile.TileContext(nc) as tc:
        with (
            tc.tile_pool(name="wpool", bufs=1) as wpool,
            tc.tile_pool(name="apool", bufs=2) as apool,
            tc.tile_pool(name="ppool", bufs=5, space="PSUM") as ppool,
        ):
            # ---- weights / constants
            w0t = wpool.tile([E, H], BF16, tag="w0t")
            nc.sync.dma_start(out=w0t, in_=d_w0[:, :])
            w1k = [wpool.tile([128, H], BF16, tag=f"w1k{k}", name=f"w1k{k}")
                   for k in range(4)]
            w2k = [wpool.tile([128, H], BF16, tag=f"w2k{k}", name=f"w2k{k}")
                   for k in range(4)]
            w3k = [wpool.tile([128, F3], BF16, tag=f"w3k{k}", name=f"w3k{k}")
                   for k in range(4)]
            for k in range(4):
                nc.sync.dma_start(out=w1k[k], in_=d_w1[128 * k:128 * (k + 1), :])
                nc.sync.dma_start(out=w2k[k], in_=d_w2[128 * k:128 * (k + 1), :])
                nc.sync.dma_start(out=w3k[k], in_=d_w3[128 * k:128 * (k + 1), :])
            b012 = wpool.tile([E, 12], F32, tag="b012")
            nc.sync.dma_start(out=b012, in_=d_b012[:, :])
            b3seg = wpool.tile([128, 4 * E], BF16, tag="b3seg")
            nc.sync.dma_start(out=b3seg, in_=d_b3seg[:, :])
            sel3 = wpool.tile([128, 3 * Nc], BF16, tag="sel3")
            nc.sync.dma_start(out=sel3, in_=d_sel3[:, :])
            bemb = wpool.tile([E, 1], F32, tag="bemb")
            nc.sync.dma_start(out=bemb, in_=d_bemb[:, :])
            wembt = wpool.tile([128, E], BF16, tag="wembt")
            nc.sync.dma_start(out=wembt, in_=d_wemb[:, :])
            xt0 = wpool.tile([128, Nc], BF16, tag="xt0")
            nc.sync.dma_start(out=xt0, in_=d_xt0[:, :])

            # ---- embed: z0 = W_embed @ x(t0) + b
            pemb = ppool.tile([E, Nc], F32, tag="pp", name="pemb")
            nc.tensor.matmul(pemb, wembt[:, :], xt0[:, :], start=True, stop=True)
            z = apool.tile([E, Nc], F32, tag="z", name="z0")
            nc.scalar.activation(z, pemb, AF.Identity, bias=bemb[:, :], scale=1.0)
            zin = apool.tile([E, Nc], BF16, tag="zin", name="zin0")
            nc.scalar.activation(zin, pemb, AF.Identity, bias=bemb[:, :], scale=1.0)

            # dx rows replicated across partitions via DMA (idle engines)
            dxb_tiles = {}
            l3_tiles = {}

            def emit_l3_bias(s):
                tiles = []
                for gi, (d0, nd) in enumerate(DX_GROUPS):
                    p3 = ppool.tile([128, nd, Nc], F32, tag="pl3", bufs=3,
                                    name=f"p3_{s}_{d0}")
                    nc.tensor.matmul(p3, b3seg[:, gi * E:(gi + 1) * E],
                                     sel3[:, 0:nd * Nc], start=True,
                                     stop=False, skip_group_check=True)
                    tiles.append(p3)
                l3_tiles[s] = tiles

            def emit_bcast(s):
                dxS = wpool.tile([E, D, Nc], BF16, tag="dxS", bufs=3,
                                 name=f"dxS_{s}")
                dap = d_dxh[:, :]
                src_bc = bass.AP(
                    tensor=dap.tensor,
                    offset=s * Nc,
                    ap=[[0, E], [N_STAGES * Nc, D], [1, Nc]])
                nc.sync.dma_start(out=dxS, in_=src_bc)
                dxb_tiles[s] = dxS

            def dxb_ap(s, d):
                return dxb_tiles[s][:, d, :]

            def relu(eng, out_ap, in_ap, bias_ap):
                if eng == "act":
                    nc.scalar.activation(out_ap, in_ap, AF.Relu,
                                         bias=bias_ap, scale=1.0)
                else:
                    e = nc.vector if eng == "dve" else nc.gpsimd
                    e.tensor_scalar(out=out_ap, in0=in_ap,
                                    scalar1=bias_ap, scalar2=0.0,
                                    op0=ALU.add, op1=ALU.max)

            def stt(eng, out_ap, in0_ap, scalar, in1_ap):
                e = nc.vector if eng == "dve" else nc.gpsimd
                e.scalar_tensor_tensor(out=out_ap, in0=in0_ap, scalar=scalar,
                                       in1=in1_ap, op0=ALU.mult, op1=ALU.add)

            RELU_ENG = [["act", "dve", "act", "dve"],
                        ["dve", "act", "dve", "act"],
                        ["act", "dve", "act", "dve"]]
            MULT_ENG = ["pool", "pool", "pool", "pool", "pool",
                        "pool", "pool", "pool", "dve", "dve"]

            emit_bcast(0)
            emit_l3_bias(0)

            def seed_p0(s, in0_ap, in1_ap):
                """p0(s) = W0^T(in0 + in1), two moving passes; the in0 pass
                runs in the previous stage's tail."""
                p0s = []
                first = in1_ap if in1_ap is not None else in0_ap
                for m in range(4):
                    p0 = ppool.tile([128, Nc], F32, tag="pp", name=f"p0_{s}_{m}")
                    nc.tensor.matmul(p0, w0t[:, 128 * m:128 * (m + 1)],
                                     first, start=True,
                                     stop=(in1_ap is None),
                                     skip_group_check=True)
                    p0s.append(p0)
                if in1_ap is not None:
                    for m in range(4):
                        nc.tensor.matmul(p0s[m], w0t[:, 128 * m:128 * (m + 1)],
                                         in0_ap, start=False, stop=True,
                                         skip_group_check=True)
                return p0s

            def vf_stage(s, p0s, coef, zbase):
                """One vector-field eval from pre-seeded L0 psums; emits the
                w = coef*(t2+r2) + zbase chain for the next stage's input."""
                y0 = apool.tile([128, 4, Nc], BF16, tag="y0", name=f"y0_{s}")
                for m in range(4):
                    relu(RELU_ENG[0][m], y0[:, m, :], p0s[m], b012[:, m:m + 1])
                # L1
                y1 = apool.tile([128, 4, Nc], BF16, tag="y1", name=f"y1_{s}")
                for m in range(4):
                    p1 = ppool.tile([128, Nc], F32, tag="pp", name=f"p1_{s}_{m}")
                    for k in range(4):
                        nc.tensor.matmul(p1, w1k[k][:, 128 * m:128 * (m + 1)],
                                         y0[:, k, :], start=(k == 0), stop=(k == 3))
                    relu(RELU_ENG[1][m], y1[:, m, :], p1, b012[:, 4 + m:5 + m])
                # L2
                y2 = apool.tile([128, 4, Nc], BF16, tag="y2", name=f"y2_{s}")
                for m in range(4):
                    p2 = ppool.tile([128, Nc], F32, tag="pp", name=f"p2_{s}_{m}")
                    for k in range(4):
                        nc.tensor.matmul(p2, w2k[k][:, 128 * m:128 * (m + 1)],
                                         y1[:, k, :], start=(k == 0), stop=(k == 3))
                    relu(RELU_ENG[2][m], y2[:, m, :], p2, b012[:, 8 + m:9 + m])
                # L3 per 3-d group: bias matmul seeds psum, 4k accumulate,
                # one tanh per group into contiguous y3all
                y3all = apool.tile([128, D, Nc], BF16, tag="y3a", name=f"y3a_{s}")
                mgs = []
                for gi, (d0, nd) in enumerate(DX_GROUPS):
                    p3 = l3_tiles[s][gi]
                    for i in range(nd):
                        d = d0 + i
                        for k in range(4):
                            nc.tensor.matmul(p3[:, i, :],
                                             w3k[k][:, 128 * d:128 * (d + 1)],
                                             y2[:, k, :], start=False,
                                             stop=(k == 3), skip_group_check=True)
                    nc.scalar.activation(y3all[:, d0:d0 + nd, :], p3, AF.Tanh)
                    # einsum partials as soon as each tanh lands (DVE):
                    # mg = y3*dx for this group; running sums off the tail
                    dxS = dxb_tiles[s]
                    mg = apool.tile([128, nd, Nc], BF16, tag=f"mg{gi}",
                                    name=f"mg_{s}_{gi}")
                    nc.vector.tensor_tensor(out=mg, in0=y3all[:, d0:d0 + nd, :],
                                            in1=dxS[:, d0:d0 + nd, :],
                                            op=ALU.mult)
                    mgs.append(mg)
                    if gi == 1:
                        s01 = apool.tile([128, 3, Nc], F32, tag="s01",
                                         name=f"s01_{s}")
                        nc.vector.tensor_tensor(out=s01, in0=mgs[0], in1=mgs[1],
                                                op=ALU.add)
                        t1 = apool.tile([128, Nc], F32, tag="t1", name=f"t1_{s}")
                        nc.vector.tensor_tensor(out=t1, in0=s01[:, 0, :],
                                                in1=s01[:, 1, :], op=ALU.add)
                        t2 = apool.tile([128, Nc], F32, tag="t2", name=f"t2_{s}")
                        nc.vector.tensor_tensor(out=t2, in0=t1, in1=s01[:, 2, :],
                                                op=ALU.add)
                        q = apool.tile([E, Nc], F32, tag="q", name=f"q_{s}")
                        stt("dve", q, t2, coef, zbase)
                    elif gi == 2:
                        r1 = apool.tile([128, Nc], F32, tag="r1", name=f"r1_{s}")
                        nc.vector.tensor_tensor(out=r1, in0=mgs[2][:, 0, :],
                                                in1=mgs[2][:, 1, :], op=ALU.add)
                        r2 = apool.tile([128, Nc], F32, tag="r2", name=f"r2_{s}")
                        nc.vector.tensor_tensor(out=r2, in0=r1,
                                                in1=mgs[2][:, 2, :], op=ALU.add)
                        w = apool.tile([E, Nc], BF16, tag="w", name=f"w_{s}")
                        stt("dve", w, r2, coef, q)
                # PE fillers for the tail: next stage's dx DMA + L3 bias seeds
                if s + 1 < N_STAGES:
                    emit_bcast(s + 1)
                    emit_l3_bias(s + 1)
                return t2, r2, mgs[3][:, 0, :], w

            def tail(s, t2, r2, mg3, w, coef):
                """Seed p0(s+1); reconstruct v2/k off-crit."""
                p0s = seed_p0(s + 1, w, mg3)
                v2 = apool.tile([E, Nc], F32, tag="v2", name=f"v2_{s}")
                nc.vector.tensor_tensor(out=v2, in0=t2, in1=r2, op=ALU.add)
                k_s = apool.tile([E, Nc], F32,
                                 tag=("k1" if s % 4 == 0 else "ks"),
                                 name=f"k_{s}")
                stt("dve", k_s, mg3, 1.0 / coef, v2)  # off-crit
                return p0s, k_s

            p0s = seed_p0(0, zin, None)
            for j in range(N_STEPS):
                hs = float(h[j])
                last = j == N_STEPS - 1

                t2, r2, mg3, w = vf_stage(4 * j + 0, p0s, hs / 3.0, z)
                p0s, k1 = tail(4 * j, t2, r2, mg3, w, hs / 3.0)
                zpart3 = apool.tile([E, Nc], F32, tag="zp3", name=f"zp3_{j}")
                stt("dve", zpart3, k1, -hs / 3.0, z)
                zacc = apool.tile([E, Nc], F32, tag="za", name=f"za1_{j}")
                stt("dve", zacc, k1, hs / 8.0, z)

                t2, r2, mg3, w = vf_stage(4 * j + 1, p0s, hs, zpart3)
                p0s, k2 = tail(4 * j + 1, t2, r2, mg3, w, hs)
                u12 = apool.tile([E, Nc], F32, tag="u12", name=f"u12_{j}")
                stt("dve", u12, k2, -1.0, k1)
                zpart4 = apool.tile([E, Nc], F32, tag="zp4", name=f"zp4_{j}")
                stt("dve", zpart4, u12, hs, z)
                zacc2 = apool.tile([E, Nc], F32, tag="za", name=f"za2_{j}")
                stt("dve", zacc2, k2, 3.0 * hs / 8.0, zacc)

                t2, r2, mg3, w = vf_stage(4 * j + 2, p0s, hs, zpart4)
                p0s, k3 = tail(4 * j + 2, t2, r2, mg3, w, hs)
                zacc3 = apool.tile([E, Nc], F32, tag="za", name=f"za3_{j}")
                stt("dve", zacc3, k3, 3.0 * hs / 8.0, zacc2)

                t2, r2, mg3, w = vf_stage(4 * j + 3, p0s, hs / 8.0, zacc3)
                if not last:
                    p0s, k4 = tail(4 * j + 3, t2, r2, mg3, w, hs / 8.0)
                    znew = apool.tile([E, Nc], F32, tag="z", name=f"z_{j + 1}")
                    stt("dve", znew, k4, hs / 8.0, zacc3)
                else:
                    znew = apool.tile([E, Nc], F32, tag="zfin", name=f"z_{j + 1}")
                    nc.vector.tensor_tensor(out=znew, in0=w, in1=mg3,
                                            op=ALU.add)
                z = znew

            nc.sync.dma_start(out=d_out[:, :], in_=z)
    nc.finalize()
    return nc


def _b3seg(b3p):
    out = np.zeros((128, 4 * E), np.float32)
    for gi, (d0, nd) in enumerate(DX_GROUPS):
        for i in range(nd):
            out[i, gi * E:(gi + 1) * E] = b3p[(d0 + i) * E:(d0 + i + 1) * E]
    return out


def _sel3(Nc):
    out = np.zeros((128, 3 * Nc), np.float32)
    for i in range(3):
        out[i, i * Nc:(i + 1) * Nc] = 1.0
    return out


def _enable_ldw_opt():
    from concourse import bass_utils as _bu
    if getattr(_bu, "_ldwopt_patched", False):
        return
    _orig = _bu.run_command

    def _run2(argv, **kw):
        argv = ["--enable-ldw-opt=true" if a == "--enable-ldw-opt=false" else a
                for a in argv]
        return _orig(argv, **kw)

    _bu.run_command = _run2
    _bu._ldwopt_patched = True


def _prep_host(t, x, mask, W_embed, b_embed, W0, b0, W1, b1, W2, b2, W3, b3):
    import ml_dtypes
    bf = ml_dtypes.bfloat16

    t = np.asarray(t, np.float32)
    x = np.asarray(x, np.float32)
    mask = np.asarray(mask)
    B, Amax = mask.shape
    N = B * Amax

    C60, h = spline_stage_matrix(t)
    idx = np.flatnonzero(mask.ravel())
    nact = max(1, len(idx))
    Nc = min(512, 4 * ((nact + 4 * N_CORES - 1) // (4 * N_CORES)))
    total = N_CORES * Nc
    pad = np.full(total, idx[0] if len(idx) else 0, dtype=np.int64)
    pad[:len(idx)] = idx
    xp = x.reshape(N, T, D)[pad]

    perm = w3_perm()
    shared = dict(
        b_emb=np.asarray(b_embed, np.float32).reshape(E, 1),
        w0t=np.ascontiguousarray(np.asarray(W0).T).astype(bf),
        w1t=np.ascontiguousarray(np.asarray(W1).T).astype(bf),
        w2t=np.ascontiguousarray(np.asarray(W2).T).astype(bf),
        w3pt=np.ascontiguousarray(np.asarray(W3)[perm].T).astype(bf),
        b012=np.stack([np.asarray(b, np.float32)[m * 128:(m + 1) * 128]
                       for b in (b0, b1, b2) for m in range(4)],
                      axis=1).astype(np.float32),
        b3seg=_b3seg(np.asarray(b3, np.float32)[perm]).astype(bf),
        sel3=_sel3(Nc).astype(bf),
        w_embt=np.concatenate([np.asarray(W_embed, np.float32).T,
                               np.zeros((128 - D, E), np.float32)], 0).astype(bf),
    )
    dx_all = np.einsum("st,ntd->snd", C60, xp.astype(np.float64))  # (60,tot,D)
    for s in range(N_STAGES):
        hs = float(h[s // 4])
        coef = (hs / 3.0, hs, hs, hs / 8.0)[s % 4]
        dx_all[s, :, D - 1] *= coef
    in_maps = []
    for c in range(N_CORES):
        xc = xp[c * Nc:(c + 1) * Nc]
        dxc = dx_all[:, c * Nc:(c + 1) * Nc, :]  # (60, Nc, D)
        dxh = np.ascontiguousarray(dxc.transpose(2, 0, 1).reshape(D, -1))
        in_maps.append(dict(
            dxh=dxh.astype(bf),                        # (10, 60*Nc)
            x_t0=np.concatenate([np.ascontiguousarray(xc[:, 0, :].T),
                                 np.zeros((128 - D, Nc), np.float32)],
                                0).astype(bf),
            **shared,
        ))
    return in_maps, pad, len(idx), Nc, h


def kernel(t, x, mask, W_embed, b_embed, W0, b0, W1, b1, W2, b2, W3, b3):
    global last_results
    from concourse import bass_utils
    if os.environ.get("KERNEL_LDWOPT", "0") == "1":
        _enable_ldw_opt()

    mask = np.asarray(mask)
    B, Amax = mask.shape
    N = B * Amax

    in_maps, pad, nact, Nc, h = _prep_host(
        t, x, mask, W_embed, b_embed, W0, b0, W1, b1, W2, b2, W3, b3)

    nc = build_bass2(Nc, h)
    res = bass_utils.run_bass_kernel_spmd(
        nc, in_maps, core_ids=list(range(N_CORES)))
    last_results = res

    zall = np.concatenate([r["zout"].T for r in res.results], 0)  # (total, E)
    out = np.zeros((N, E), np.float32)
    out[pad[:nact]] = zall[:nact]
    return out.reshape(B, Amax, E)
